# revision 11
# baseline (speedup 1.0000x reference)
"""DCGRU cell Trainium2 kernel (Bass/Tile), data-parallel over batch.

Sharding: batch B=32 split across 8 NeuronCores (B_local=4); supports and
weights replicated; zero cross-device communication.

Per-core layout: diffusion-state rows live in DRAM as (N, 384) bf16 where
row n = [b0: f0..f65 | b1: f0..f65 | b2 | b3 | 120 pad cols]  (b-major,
f0..f1 = input dims, f2..f65 = hidden units; 768B rows give full-rate DMA
descriptors; pad contents are never read).

spmm y = S @ x: edges sorted by destination; per 128-edge chunk a dma_gather
pulls the 128 source rows into SBUF partitions; a PE matmul with a host-built
(128, 32) value matrix scatter-accumulates the chunk into the aligned-32
destination window of a (128, 512) PSUM tile.  The Chebyshev step 2
(2*S@x1 - x0) folds the 2x into the values and the -x0 into the PSUM
evacuation.

Projection per 128-node tile: PE transposes per (matrix, batch) of the
contiguous (128, 66) column slices bring f onto partitions (stacked 4-wide in
PSUM free space), then 5 accumulating W matmuls with N = 4*128 (batch, node)
produce the gconv output; ACT applies bias+sigmoid/tanh; DVE runs the GRU
elementwise tail.
"""
import os
import numpy as np
import ml_dtypes
from contextlib import ExitStack

import concourse.bass as bass
import concourse.tile as tile
from concourse import bacc, mybir
from concourse.bass_utils import run_bass_kernel_spmd
from concourse.masks import make_identity

N = 8000
DEG = 8
B = 32
IN_DIM = 2
UNITS = 64
NCORES = 8
BL = B // NCORES          # 4
F = IN_DIM + UNITS        # 66
NMAT = 5                  # 2*K+1
ROW = F * BL              # 264 used cols
EROW = 384                # padded bf16 row
H1 = 2 * UNITS
H2 = UNITS
NT = (N + 127) // 128     # 63 (last tile 64 rows)
E = N * DEG               # 64000
NCH = E // 128            # 500
CALL_CHUNKS = 8

BF16 = ml_dtypes.bfloat16
DT = mybir.dt.bfloat16
F32 = mybir.dt.float32


def _set_dims(n=None, deg=None, b=None, ncores=None):
    """Dev helper: reconfigure problem dims (for scaled-down sim tests)."""
    global N, DEG, B, NCORES, BL, ROW, NT, E, NCH
    if n is not None:
        N = n
    if deg is not None:
        DEG = deg
    if b is not None:
        B = b
    if ncores is not None:
        NCORES = ncores
    BL = B // NCORES
    ROW = F * BL
    NT = (N + 127) // 128
    E = N * DEG
    NCH = E // 128


def _tile_rows(t):
    return 128 if t < NT - 1 else N - 128 * (NT - 1)


# ---------------------------------------------------------------- host prep
def _prep_support(rows, cols, vals):
    """Dest-sorted edges -> gather idx + scatter matmul schedule.

    Returns (idx_wrapped (128, E/16) int16, mms, lhsT (128, n_mm*32) f32).
    mms[i] = [chunk_j, tile_t, off, start, stop, evac_after]
    """
    assert rows.shape[0] == E
    order = np.argsort(rows, kind="stable")
    d = rows[order].astype(np.int64)
    s = cols[order].astype(np.int64)
    v = vals[order].astype(np.float64)
    mms = []
    lhs = []
    win_first = {}
    win_last = {}
    tile_last = {}
    for j in range(NCH):
        dd = d[j * 128:(j + 1) * 128]
        vv = v[j * 128:(j + 1) * 128]
        for w in np.unique(dd // 32):
            w = int(w)
            t, off = w // 4, (w % 4) * 32
            lh = np.zeros((128, 32), np.float32)
            m = (dd // 32) == w
            lh[np.nonzero(m)[0], dd[m] - w * 32] = vv[m]
            i = len(mms)
            mms.append([j, t, off, False, False, False])
            lhs.append(lh)
            if w not in win_first:
                win_first[w] = i
            win_last[w] = i
            tile_last[t] = i
    nwin = (N + 31) // 32
    assert set(win_first.keys()) == set(range(nwin)), "empty 32-row window"
    for w, i in win_first.items():
        mms[i][3] = True
    for w, i in win_last.items():
        mms[i][4] = True
    for t, i in tile_last.items():
        mms[i][5] = True
    lhsT = np.concatenate(lhs, axis=1)
    idxw = np.tile(s.reshape(-1, 16).T.astype(np.int16), (8, 1))
    return idxw, mms, lhsT


def _build_rows(x_nbf):
    """x (N, BL, F) f32 -> (N, EROW) bf16 b-major rows."""
    out = np.zeros((N, EROW), BF16)
    out[:, :ROW] = x_nbf.reshape(N, ROW).astype(BF16)
    return out


def _expand_w(Wf):
    """W (F*NMAT, H) -> per-matrix (F, H) bf16 blocks (rows f0..f65)."""
    H = Wf.shape[1]
    out = []
    for m in range(NMAT):
        blk = np.zeros((F, H), np.float32)
        for f in range(F):
            blk[f] = Wf[f * NMAT + m]
        out.append(blk.astype(BF16))
    return out


_NMM = {}
_MMS = {}
_WRU = None
_WC = None


# ---------------------------------------------------------------- bass build
def _emit_spmm(nc, pools, src_dram, dst_dram, idx_t, lhsT_dram, mms,
               x0_big, subtract):
    """dst = S @ src (or 2*S@src - x0 with values pre-scaled by 2)."""
    xg_pool, lhs_pool, ps_pool, ev_pool = pools
    mm_by_chunk = [[] for _ in range(NCH)]
    for i, (j, t, off, st, sp, ev) in enumerate(mms):
        mm_by_chunk[j].append((i, t, off, st, sp, ev))

    ps_tiles = {}
    ncalls = (NCH + CALL_CHUNKS - 1) // CALL_CHUNKS
    for call in range(ncalls):
        c0 = call * CALL_CHUNKS
        c1 = min(NCH, c0 + CALL_CHUNKS)
        ncc = c1 - c0
        nidx = ncc * 128
        xg = xg_pool.tile([128, CALL_CHUNKS, EROW], DT, tag="xg")
        nc.gpsimd.dma_gather(
            out_ap=xg[:, 0:ncc, :],
            in_ap=src_dram[:],
            idxs_ap=idx_t[:, c0 * 8:c0 * 8 + nidx // 16],
            num_idxs=nidx,
            num_idxs_reg=nidx,
            elem_size=EROW,
        )
        i_lo = mm_by_chunk[c0][0][0]
        i_hi = mm_by_chunk[c1 - 1][-1][0] + 1
        lh = lhs_pool.tile([128, (i_hi - i_lo) * 32], DT, tag="lh")
        nc.sync.dma_start(lh[:], lhsT_dram[:, i_lo * 32:i_hi * 32])
        for c in range(c0, c1):
            rhs = xg[:, c - c0, 0:ROW]
            for (i, t, off, st, sp, ev) in mm_by_chunk[c]:
                pt = ps_tiles.get(t)
                if pt is None:
                    pt = ps_pool.tile([128, 512], F32, space="PSUM", tag="scat")
                    ps_tiles[t] = pt
                nc.tensor.matmul(
                    out=pt[off:off + 32, 0:ROW],
                    lhsT=lh[:, (i - i_lo) * 32:(i - i_lo) * 32 + 32],
                    rhs=rhs,
                    start=st, stop=sp,
                    tile_position=(0, off),
                    skip_group_check=True,
                )
                if ev:
                    _evac_tile(nc, ev_pool, pt, t, dst_dram, x0_big, subtract)
                    del ps_tiles[t]
    assert not ps_tiles


def _evac_tile(nc, ev_pool, pt, t, dst_dram, x0_big, subtract):
    rows = _tile_rows(t)
    ev = ev_pool.tile([128, ROW], DT, tag="ev")
    if subtract:
        nc.vector.tensor_sub(ev[0:rows, :], pt[0:rows, 0:ROW],
                             x0_big[0:rows, t * ROW:(t + 1) * ROW])
    else:
        nc.scalar.copy(ev[0:rows, :], pt[0:rows, 0:ROW])
    nc.sync.dma_start(dst_dram[t * 128:t * 128 + rows, 0:ROW], ev[0:rows, :])


def _load_big(nc, big_t, dram):
    """DRAM (N, EROW)[:, 0:ROW] -> SBUF (128, NT*ROW); node n -> (n%128, n//128)."""
    nfull = NT - 1
    src = dram[0:nfull * 128, 0:ROW].rearrange("(t p) e -> p t e", p=128)
    dst = big_t[:, 0:nfull * ROW].rearrange("p (t e) -> p t e", t=nfull)
    nc.sync.dma_start(dst, src)
    rows = N - nfull * 128
    nc.sync.dma_start(big_t[0:rows, nfull * ROW:NT * ROW],
                      dram[nfull * 128:N, 0:ROW])


def _build_program():
    nc = bacc.Bacc("TRN2", target_bir_lowering=False, debug=False)

    def din(name, shape, dt=DT):
        return nc.dram_tensor(name, shape, dt, kind="ExternalInput").ap()

    x0_d = din("x0", (N, EROW))
    idx_d = [din(f"idx{s}", (128, E // 16), mybir.dt.int16) for s in range(2)]
    ls_d = {(s, step): din(f"ls{s}_{step}", (128, _NMM[s] * 32))
            for s in range(2) for step in (1, 2)}
    wru_d = [din(f"wru{m}", (F, H1)) for m in range(NMAT)]
    wc_d = [din(f"wc{m}", (F, H2)) for m in range(NMAT)]
    bru_d = din("bru", (H1, 1), F32)
    bc_d = din("bc", (H2, 1), F32)
    hxT_d = din("hxT", (BL, UNITS, N), F32)
    out_d = nc.dram_tensor("out", (BL, UNITS, N), F32, kind="ExternalOutput").ap()

    x1 = [nc.dram_tensor(f"x1_{s}", (N, EROW), DT).ap() for s in range(2)]
    x2 = [nc.dram_tensor(f"x2_{s}", (N, EROW), DT).ap() for s in range(2)]
    x1b = [nc.dram_tensor(f"x1b_{s}", (N, EROW), DT).ap() for s in range(2)]
    x2b = [nc.dram_tensor(f"x2b_{s}", (N, EROW), DT).ap() for s in range(2)]
    x0p_d = nc.dram_tensor("x0p", (N, EROW), DT).ap()
    u_d = nc.dram_tensor("u", (BL, UNITS, N), F32).ap()

    with tile.TileContext(nc) as tc, ExitStack() as ctx:
        const = ctx.enter_context(tc.tile_pool(name="const", bufs=1))
        big = ctx.enter_context(tc.tile_pool(name="big", bufs=1))
        xg_pool = ctx.enter_context(tc.tile_pool(name="xg", bufs=3))
        lhs_pool = ctx.enter_context(tc.tile_pool(name="lhs", bufs=2))
        ps_pool = ctx.enter_context(tc.tile_pool(name="ps", bufs=2, space="PSUM"))
        ev_pool = ctx.enter_context(tc.tile_pool(name="ev", bufs=3))
        ph3 = ctx.enter_context(tc.tile_pool(name="ph3", bufs=3))
        tp_ps = ctx.enter_context(tc.tile_pool(name="tp", bufs=2, space="PSUM"))
        wm_ps = ctx.enter_context(tc.tile_pool(name="wm", bufs=2, space="PSUM"))
        rh_ps = ctx.enter_context(tc.tile_pool(name="rh", bufs=2, space="PSUM"))
        spools = (xg_pool, lhs_pool, ps_pool, ev_pool)

        ident = const.tile([128, 128], DT)
        make_identity(nc, ident[:])

        idx_t = []
        for s in range(2):
            it = const.tile([128, E // 16], mybir.dt.int16, tag=f"idx{s}")
            nc.sync.dma_start(it[:], idx_d[s][:])
            idx_t.append(it)
        wru_t = []
        for m in range(NMAT):
            t_ = const.tile([F, H1], DT, tag=f"wru{m}")
            nc.sync.dma_start(t_[:], wru_d[m][:])
            wru_t.append(t_)
        wc_t = []
        for m in range(NMAT):
            t_ = const.tile([F, H2], DT, tag=f"wc{m}")
            nc.sync.dma_start(t_[:], wc_d[m][:])
            wc_t.append(t_)
        bru_t = const.tile([H1, 1], F32)
        nc.sync.dma_start(bru_t[:], bru_d[:])
        bc_t = const.tile([H2, 1], F32)
        nc.sync.dma_start(bc_t[:], bc_d[:])

        x0_big = big.tile([128, NT * ROW], DT, tag="x0big")
        _load_big(nc, x0_big, x0_d)
        x0p_big = big.tile([128, NT * ROW], DT, tag="x0pbig")

        def gconv(x0_dram, x0_sb, xs1, xs2, wk_t, H, bias_t, act, consume):
            for s in range(2):
                _emit_spmm(nc, spools, x0_dram, xs1[s], idx_t[s],
                           ls_d[(s, 1)], _MMS[s][0], None, False)
            for s in range(2):
                _emit_spmm(nc, spools, xs1[s], xs2[s], idx_t[s],
                           ls_d[(s, 2)], _MMS[s][1], x0_sb, True)
            for t in range(NT):
                rows = _tile_rows(t)
                xs_all = ph3.tile([128, NMAT * ROW], DT, tag="xsall")
                nc.scalar.copy(xs_all[0:rows, 0:ROW],
                               x0_sb[0:rows, t * ROW:(t + 1) * ROW])
                for m, dram in ((1, xs1[0]), (2, xs2[0]), (3, xs1[1]),
                                (4, xs2[1])):
                    nc.sync.dma_start(xs_all[0:rows, m * ROW:m * ROW + ROW],
                                      dram[t * 128:t * 128 + rows, 0:ROW])
                po = wm_ps.tile([128, BL * 128], F32, space="PSUM", tag="wmm")
                for m in range(NMAT):
                    # transpose the 4 per-batch (rows, 66) slices of matrix m
                    tp = tp_ps.tile([128, BL * 128], DT, space="PSUM", tag="tp")
                    for b in range(BL):
                        nc.tensor.transpose(
                            out=tp[0:F, b * 128:b * 128 + rows],
                            in_=xs_all[0:rows, m * ROW + b * F:m * ROW + (b + 1) * F],
                            identity=ident[0:rows, 0:rows])
                    st = ph3.tile([F, BL * 128], DT, tag="xsT")
                    if m % 2 == 0:
                        nc.scalar.copy(st[:], tp[0:F, :])
                    else:
                        nc.vector.tensor_copy(st[:], tp[0:F, :])
                    nc.tensor.matmul(
                        out=po[0:H, :],
                        lhsT=wk_t[m][:, 0:H],
                        rhs=st[:],
                        start=(m == 0), stop=(m == NMAT - 1),
                    )
                act_t = ph3.tile([128, BL * 128], F32, tag="act")
                nc.scalar.activation(act_t[0:H, :], po[0:H, :], act,
                                     bias=bias_t[:])
                consume(t, act_t, rows)

        def consume1(t, act_t, rows):
            # act_t (H1, (b, n)): r = [0:64], u = [64:128]
            for b in range(BL):
                nc.sync.dma_start(u_d[b, :, t * 128:t * 128 + rows],
                                  act_t[UNITS:H1, b * 128:b * 128 + rows])
            hx = ph3.tile([UNITS, BL * 128], F32, tag="hx")
            for b in range(BL):
                nc.sync.dma_start(hx[:, b * 128:b * 128 + rows],
                                  hxT_d[b, :, t * 128:t * 128 + rows])
            rh = ph3.tile([UNITS, BL * 128], DT, tag="rh")
            nc.vector.tensor_mul(rh[:], act_t[0:UNITS, :], hx[:])
            for b in range(BL):
                tp = rh_ps.tile([128, UNITS], DT, space="PSUM", tag="rhT")
                nc.tensor.transpose(out=tp[0:rows, :],
                                    in_=rh[:, b * 128:b * 128 + rows],
                                    identity=ident[0:UNITS, 0:UNITS])
                nc.vector.tensor_copy(
                    x0p_big[0:rows,
                            t * ROW + b * F + IN_DIM:t * ROW + (b + 1) * F],
                    tp[0:rows, :])

        gconv(x0_d, x0_big, x1, x2, wru_t, H1, bru_t,
              mybir.ActivationFunctionType.Sigmoid, consume1)

        # finish x0p rows: xi cols from x0, then write to DRAM
        xiv_src = x0_big[:].rearrange("p (t b f) -> p t b f", t=NT, b=BL)
        xiv_dst = x0p_big[:].rearrange("p (t b f) -> p t b f", t=NT, b=BL)
        nc.vector.tensor_copy(xiv_dst[:, :, :, 0:IN_DIM],
                              xiv_src[:, :, :, 0:IN_DIM])
        for t in range(NT):
            rows = _tile_rows(t)
            nc.sync.dma_start(x0p_d[t * 128:t * 128 + rows, 0:ROW],
                              x0p_big[0:rows, t * ROW:(t + 1) * ROW])

        def consume2(t, act_t, rows):
            hx = ph3.tile([UNITS, BL * 128], F32, tag="hx2")
            ut = ph3.tile([UNITS, BL * 128], F32, tag="ut")
            for b in range(BL):
                nc.sync.dma_start(hx[:, b * 128:b * 128 + rows],
                                  hxT_d[b, :, t * 128:t * 128 + rows])
                nc.sync.dma_start(ut[:, b * 128:b * 128 + rows],
                                  u_d[b, :, t * 128:t * 128 + rows])
            d1 = ph3.tile([UNITS, BL * 128], F32, tag="d1")
            nc.vector.tensor_sub(d1[:], hx[:], act_t[0:H2, :])
            d2 = ph3.tile([UNITS, BL * 128], F32, tag="d2")
            nc.vector.tensor_mul(d2[:], ut[:], d1[:])
            ot = ph3.tile([UNITS, BL * 128], F32, tag="ot")
            nc.vector.tensor_add(ot[:], d2[:], act_t[0:H2, :])
            for b in range(BL):
                nc.sync.dma_start(out_d[b, :, t * 128:t * 128 + rows],
                                  ot[:, b * 128:b * 128 + rows])

        gconv(x0p_d, x0p_big, x1b, x2b, wc_t, H2, bc_t,
              mybir.ActivationFunctionType.Tanh, consume2)

    nc.compile()
    return nc


# ---------------------------------------------------------------- entry
def kernel(inputs, hx, W_ru, b_ru, W_c, b_c,
           s0_rows, s0_cols, s0_vals, s1_rows, s1_cols, s1_vals):
    global _WRU, _WC, _LAST_NC, _LAST_IN_MAPS
    inputs = np.asarray(inputs, np.float32)
    hx = np.asarray(hx, np.float32)
    supports = [
        (np.asarray(s0_rows), np.asarray(s0_cols),
         np.asarray(s0_vals, np.float32)),
        (np.asarray(s1_rows), np.asarray(s1_cols),
         np.asarray(s1_vals, np.float32)),
    ]
    idxw, lhsT1, lhsT2 = [], [], []
    for s, (r, c, v) in enumerate(supports):
        iw, mms1, lh1 = _prep_support(r, c, v)
        _, mms2, lh2 = _prep_support(r, c, 2.0 * v)
        _NMM[s] = lh1.shape[1] // 32
        _MMS[s] = (mms1, mms2)
        idxw.append(iw)
        lhsT1.append(lh1.astype(BF16))
        lhsT2.append(lh2.astype(BF16))
    _WRU = _expand_w(np.asarray(W_ru, np.float32))
    _WC = _expand_w(np.asarray(W_c, np.float32))

    nc = _build_program()

    xin = inputs.reshape(B, N, IN_DIM)
    hxr = hx.reshape(B, N, UNITS)
    in_maps = []
    for core in range(NCORES):
        bs = slice(core * BL, (core + 1) * BL)
        # x rows: (N, BL, F) with F = [xi(2), units(64)]
        xbf = np.concatenate([xin[bs], hxr[bs]], axis=2)   # (BL, N, F)
        xbf = np.transpose(xbf, (1, 0, 2))                 # (N, BL, F)
        m = {
            "x0": _build_rows(xbf),
            "idx0": idxw[0], "idx1": idxw[1],
            "ls0_1": lhsT1[0], "ls0_2": lhsT2[0],
            "ls1_1": lhsT1[1], "ls1_2": lhsT2[1],
            "bru": np.asarray(b_ru, np.float32).reshape(H1, 1),
            "bc": np.asarray(b_c, np.float32).reshape(H2, 1),
            "hxT": np.ascontiguousarray(
                np.transpose(hxr[bs], (0, 2, 1))).astype(np.float32),
        }
        for k, t in enumerate(_WRU):
            m[f"wru{k}"] = t
        for k, t in enumerate(_WC):
            m[f"wc{k}"] = t
        in_maps.append(m)

    _LAST_NC, _LAST_IN_MAPS = nc, in_maps
    if os.environ.get("KERNEL_SIM"):
        from concourse.bass_interp import CoreSim
        assert NCORES == 1
        sim = CoreSim(nc)
        for k, v in in_maps[0].items():
            sim.tensor(k)[:] = v
        # pre-zero internal DRAM so pad-column reads are initialized in sim
        for name in (["x0p", "u"] + [f"x{i}_{s}" for i in (1, 2)
                                     for s in range(2)]
                     + [f"x{i}b_{s}" for i in (1, 2) for s in range(2)]):
            sim.tensor(name)[:] = 0
        sim.simulate()
        results = [{"out": np.array(sim.tensor("out"))}]
    else:
        res = run_bass_kernel_spmd(nc, in_maps, core_ids=list(range(NCORES)))
        results = res.results
    outs = []
    for core in range(NCORES):
        o = np.asarray(results[core]["out"])          # (BL, 64, N)
        outs.append(np.transpose(o, (0, 2, 1)).reshape(BL, N * UNITS))
    return np.concatenate(outs, axis=0).astype(np.float32)
